# revision 1
# baseline (speedup 1.0000x reference)
"""Trainium2 Bass kernel for nn_MultiHeadAttention_8074538516581.

Sharding: 8 cores = batch(4) x head-group(2 groups of 6 heads).
Each core computes, for its (b, g):
  qkv slice projection (bf16 matmuls, fp32 psum accum, struct-embed term
  folded in as a rank-4 matmul O @ (SE @ W^T)), per-head attention with the
  reference's exact semantics (q/k rounded to bf16, fixed-shift-free softmax
  -- the row-max subtraction cancels in the normalization, the [-30,30] logit
  clip and the 1e5/1e-10 guards are provably inactive here), and the partial
  output projection over its 384 head-dims.
Host sums the two head-group partials per batch and adds b_out.

Token permutation: queries with (t % 64) % 3 == 0 are zeroed by the
reference's load mask, making their attention output mean(v) per head.
Tokens are permuted live-first so the 672 live queries are contiguous:
scores/exp/pv run only on live columns; the 352 masked columns get the
per-head mean(v) via one N=1 matmul + broadcast.
"""
import numpy as np
import ml_dtypes

import concourse.bass as bass
import concourse.mybir as mybir
import concourse.tile as tile
from concourse import bacc
from concourse.bass import ts
from concourse.bass_utils import run_bass_kernel_spmd

B, T, E = 4, 1024, 768
H, D = 12, 64
HG = 6                  # heads per group
GD = HG * D             # 384 head-dims per group
BLOCK_M = 64
LIVE = 672              # tokens with (t % BLOCK_M) % 3 != 0
MASK = T - LIVE         # 352
SCALE = 1.0 / 8.0       # 1/sqrt(64)

BF16 = mybir.dt.bfloat16
F32 = mybir.dt.float32

_perm = None
_nc = None


def _perm_live_first():
    t = np.arange(T)
    m = (t % BLOCK_M) % 3 == 0
    return np.concatenate([t[~m], t[m]])


def _build_bass(debug=False, repeat=1, upto="full"):
    nc = bacc.Bacc()
    xT_d = nc.dram_tensor("xT", [E, T], BF16, kind="ExternalInput")
    wT_d = nc.dram_tensor("wT", [E, 3 * GD], BF16, kind="ExternalInput")
    ot_d = nc.dram_tensor("ot", [4, T], BF16, kind="ExternalInput")
    m2_d = nc.dram_tensor("m2", [4, 3 * GD], BF16, kind="ExternalInput")
    woT_d = nc.dram_tensor("woT", [GD, E], BF16, kind="ExternalInput")
    out_d = nc.dram_tensor("out", [T, E], F32, kind="ExternalOutput")

    dbg_p = None
    if debug:
        dbg_p = nc.dram_tensor("dbg_p", [128, 8, LIVE], BF16, kind="ExternalOutput")
        dbg_s = nc.dram_tensor("dbg_s", [128, 8, LIVE], F32, kind="ExternalOutput")

    from contextlib import ExitStack
    with tile.TileContext(nc) as tc, ExitStack() as rep_ctx:
        with tc.tile_pool(name="singles", bufs=1) as singles:
            xT_sb = singles.tile([128, 6, T], BF16)
            wT_sb = singles.tile([128, 6, 3 * GD], BF16)
            woT_sb = singles.tile([128, 3, E], BF16)
            ot_sb = singles.tile([4, T], BF16)
            m2_sb = singles.tile([4, 3 * GD], BF16)
            ones_p = singles.tile([128, MASK], BF16)
            qT_sb = singles.tile([128, 3, T], BF16)   # cols LIVE: garbage, never read
            kT_sb = singles.tile([128, 3, T], BF16)
            v_sb = singles.tile([128, 8, HG * (D + 1)], BF16)  # per-head v | ones col
            attnT_sb = singles.tile([128, 3, T], BF16)

            nc.sync.dma_start(out=xT_sb, in_=xT_d[:, :].rearrange("(c p) t -> p c t", p=128))
            nc.sync.dma_start(out=wT_sb, in_=wT_d[:, :].rearrange("(c p) t -> p c t", p=128))
            nc.sync.dma_start(out=woT_sb, in_=woT_d[:, :].rearrange("(c p) t -> p c t", p=128))
            nc.sync.dma_start(out=ot_sb, in_=ot_d[:, :])
            nc.sync.dma_start(out=m2_sb, in_=m2_d[:, :])
            nc.vector.memset(ones_p, 1.0)
            v_ones = v_sb[:, :, :].rearrange("p a (h e) -> p a h e", e=D + 1)[:, :, :, D:D + 1]
            nc.vector.memset(v_ones, 1.0)
            # q column LIVE is pinned to 0 so exp gives p'=1 there: the pv
            # matmul's column LIVE-512 then lands [sum(v) | 1024] = the
            # masked-query numerator and denominator, with a single
            # start=True writer chain per PSUM bank.
            nc.vector.memset(qT_sb[:, :, LIVE:LIVE + 1], 0.0)

            if repeat > 1:
                rep_ctx.enter_context(tc.For_i(0, repeat, 1))

            # ---- Phase 1: v projection (natural layout, feeds all heads)
            with tc.tile_pool(name="v_ps", bufs=2, space="PSUM") as v_pool:
                for tt in range(8 if upto != "dma" else 0):
                    ps = v_pool.tile([128, GD], F32, tag="vps")
                    for ek in range(6):
                        nc.tensor.matmul(ps,
                                         xT_sb[:, ek, ts(tt, 128)],
                                         wT_sb[:, ek, 2 * GD:3 * GD],
                                         start=(ek == 0), stop=False)
                    nc.tensor.matmul(ps, ot_sb[:, ts(tt, 128)],
                                     m2_sb[:, 2 * GD:3 * GD], start=False, stop=True)
                    dst = v_sb[:, tt, :].rearrange("p (h e) -> p h e", e=D + 1)[:, :, 0:D]
                    src = ps[:, :].rearrange("p (h d) -> p h d", d=D)
                    nc.scalar.copy(dst, src)

            # ---- Phase 2: per head-pair: project q,k chunk then attend both
            # heads. Keeps PE dense (projection of pair c+1 overlaps the
            # ACT-bound softmax of pair c) so HAM stays warm.
            with tc.tile_pool(name="sT_ps", bufs=2, space="PSUM") as sT_pool, \
                 tc.tile_pool(name="acc_ps", bufs=2, space="PSUM") as acc_pool, \
                 tc.tile_pool(name="pp", bufs=3) as pp_pool, \
                 tc.tile_pool(name="sm", bufs=3) as sm_pool, \
                 tc.tile_pool(name="dscr", bufs=3, space="DRAM") as dr_pool:
                for c in range(3 if upto not in ("dma", "v") else 0):
                    for mt in (c, c + 3):    # q chunk then k chunk
                        ps = sT_pool.tile([128, T], F32, tag="sT", name="qkps")
                        isq = mt < 3
                        slices = ((0, 512), (512, LIVE)) if isq else ((0, 512), (512, T))
                        for ek in range(6):
                            for s0, s1 in slices:
                                nc.tensor.matmul(ps[:, s0:s1],
                                                 wT_sb[:, ek, ts(mt, 128)],
                                                 xT_sb[:, ek, s0:s1],
                                                 start=(ek == 0), stop=False)
                        for s0, s1 in slices:
                            nc.tensor.matmul(ps[:, s0:s1],
                                             m2_sb[:, ts(mt, 128)],
                                             ot_sb[:, s0:s1],
                                             start=False, stop=True)
                        if isq:
                            nc.vector.tensor_copy(qT_sb[:, mt, 0:LIVE], ps[:, 0:LIVE])
                        else:
                            nc.vector.tensor_copy(kT_sb[:, mt - 3, :], ps[:, :])

                    for h in (2 * c, 2 * c + 1):
                        po = (h % 2) * 64
                        qh = qT_sb[po:po + 64, c, :]
                        kh = kT_sb[po:po + 64, c, :]
                        acc1 = acc_pool.tile([65, T], F32, tag="acc1")
                        for kt in range(8):
                            # [0:512) in bank 0, [512:673) in bank 1, aligned
                            sT = sT_pool.tile([128, T], F32, tag="sT")
                            pp = pp_pool.tile([128, LIVE + 1], BF16, tag="pp")
                            nc.tensor.matmul(sT[:, 0:512], kh[:, ts(kt, 128)],
                                             qh[:, 0:512], start=True, stop=True)
                            nc.tensor.matmul(sT[:, 512:LIVE + 1], kh[:, ts(kt, 128)],
                                             qh[:, 512:LIVE + 1], start=True, stop=True)
                            nc.scalar.activation(pp[:, 0:LIVE + 1], sT[:, 0:LIVE + 1],
                                                 mybir.ActivationFunctionType.Exp,
                                                 scale=SCALE)
                            if debug and h == 0:
                                nc.sync.dma_start(out=dbg_p[:, kt, :], in_=pp[:, 0:LIVE])
                                sc = sm_pool.tile([128, LIVE], F32, tag="dbgsc")
                                nc.vector.tensor_copy(sc, sT[:, 0:LIVE])
                                nc.sync.dma_start(out=dbg_s[:, kt, :], in_=sc)
                            vh = v_sb[:, kt, h * (D + 1):(h + 1) * (D + 1)]
                            nc.tensor.matmul(acc1[:, 0:512], vh, pp[:, 0:512],
                                             start=(kt == 0), stop=(kt == 7))
                            nc.tensor.matmul(acc1[:, 512:LIVE + 1], vh,
                                             pp[:, 512:LIVE + 1],
                                             start=(kt == 0), stop=(kt == 7))
                        # normalize by denominators (row 64); acc is
                        # double-buffered so this chain overlaps the next head
                        rd = sm_pool.tile([1, LIVE + 1], F32, tag="rd")
                        nc.vector.reciprocal(rd, acc1[64:65, 0:LIVE + 1])
                        # partition-broadcast via DRAM roundtrip (DMA can
                        # replicate from linear memory; SBUF-source
                        # zero-stride partition APs are not allowed)
                        rb = sm_pool.tile([64, LIVE + 1], F32, tag="rb")
                        dscr = dr_pool.tile([1, LIVE + 1], F32, tag="dscr")
                        nc.sync.dma_start(out=dscr, in_=rd[0:1, :])
                        src = dscr[0:1, :]
                        bc = bass.AP(tensor=src.tensor, offset=src.offset,
                                     ap=[[0, 64]] + [list(a) for a in src.ap[1:]])
                        nc.sync.dma_start(out=rb, in_=bc)
                        ah = attnT_sb[po:po + 64, c, :]
                        nc.vector.tensor_mul(ah[:, 0:LIVE], acc1[0:64, 0:LIVE],
                                             rb[:, 0:LIVE])
                        mv = sm_pool.tile([64, 1], F32, tag="mv")
                        nc.vector.tensor_scalar_mul(mv, acc1[0:64, LIVE:LIVE + 1],
                                                    rb[0:64, LIVE:LIVE + 1])
                        nc.vector.tensor_scalar_mul(ah[:, LIVE:T], ones_p[0:64, :], mv)

            if debug:
                for nm, t, sh in (("dbg_q", qT_sb, [128, 3, T]),
                                  ("dbg_k", kT_sb, [128, 3, T]),
                                  ("dbg_v", v_sb, [128, 8, HG * (D + 1)]),
                                  ("dbg_a", attnT_sb, [128, 3, T])):
                    dd = nc.dram_tensor(nm, sh, BF16, kind="ExternalOutput")
                    nc.sync.dma_start(out=dd[:, :, :], in_=t[:, :, :])

            # ---- Phase 3: output projection (partial over this group's dims)
            ob_singles = None
            if repeat > 1:
                ob_singles = []
                for i in range(8):
                    obs = singles.tile([128, E], F32, tag=f"obs{i}", name=f"obs{i}")
                    ob_singles.append(obs)
            with tc.tile_pool(name="o_ps", bufs=3, space="PSUM") as o_pool, \
                 tc.tile_pool(name="ob", bufs=3) as ob_pool:
                for tt in range(8 if upto == "full" else 0):
                    ps = o_pool.tile([128, E], F32, tag="ops")
                    for s0, s1 in ((0, 512), (512, E)):
                        for c3 in range(3):
                            nc.tensor.matmul(ps[:, s0:s1],
                                             attnT_sb[:, c3, ts(tt, 128)],
                                             woT_sb[:, c3, s0:s1],
                                             start=(c3 == 0), stop=(c3 == 2))
                    if repeat > 1:
                        nc.vector.tensor_copy(ob_singles[tt], ps)
                    else:
                        ob = ob_pool.tile([128, E], F32, tag="ob")
                        nc.vector.tensor_copy(ob, ps)
                        nc.sync.dma_start(out=out_d[ts(tt, 128), :], in_=ob)
            if repeat > 1:
                rep_ctx.close()
                for tt in range(8 if upto == "full" else 0):
                    nc.sync.dma_start(out=out_d[ts(tt, 128), :], in_=ob_singles[tt])

    nc.finalize()
    return nc


def _get_bass():
    global _nc
    if _nc is None:
        _nc = _build_bass()
    return _nc


def kernel(x, idx, struct_embed, w_qkv, w_out, b_out):
    global _perm
    if _perm is None:
        _perm = _perm_live_first()
    perm = _perm

    x = np.asarray(x, dtype=np.float32)
    idx = np.asarray(idx)
    struct_embed = np.asarray(struct_embed, dtype=np.float32)
    w_qkv = np.asarray(w_qkv, dtype=np.float32)
    w_out = np.asarray(w_out, dtype=np.float32)
    b_out = np.asarray(b_out, dtype=np.float32)

    sid = ((idx == 1) * 1 + (idx == 2) * 2 + (idx == 3) * 3).astype(np.int64)  # [B,T]
    oh = (sid[:, :, None] == np.arange(4)[None, None, :]).astype(np.float32)  # [B,T,4]

    bf = ml_dtypes.bfloat16
    in_maps = []
    for core in range(8):
        b, g = core // 2, core % 2
        wg = np.concatenate([w_qkv[g * GD:(g + 1) * GD],
                             w_qkv[E + g * GD:E + (g + 1) * GD],
                             w_qkv[2 * E + g * GD:2 * E + (g + 1) * GD]], axis=0)  # [3GD, E]
        in_maps.append({
            "xT": np.ascontiguousarray(x[b].T[:, perm]).astype(bf),
            "wT": np.ascontiguousarray(wg.T).astype(bf),
            "ot": np.ascontiguousarray(oh[b].T[:, perm]).astype(bf),
            "m2": (struct_embed @ wg.T).astype(bf),
            "woT": np.ascontiguousarray(w_out[:, g * GD:(g + 1) * GD].T).astype(bf),
        })

    res = run_bass_kernel_spmd(_get_bass(), in_maps, core_ids=list(range(8)))

    inv = np.empty(T, dtype=np.int64)
    inv[perm] = np.arange(T)
    out = np.empty((B, T, E), dtype=np.float32)
    for b in range(B):
        acc = res.results[2 * b]["out"] + res.results[2 * b + 1]["out"]
        out[b] = acc[inv] + b_out[None, :]
    return out



# revision 13
# speedup vs baseline: 1.6075x; 1.6075x over previous
"""Trainium2 Bass kernel for nn_MultiHeadAttention_8074538516581.

Sharding: 8 cores = batch(4) x head-group(2 groups of 6 heads).
Each core computes, for its (b, g): qkv projection for its 6 heads
(struct-embed folded into x on the host), per-head attention with the
reference's exact semantics (q/k rounded to bf16; the row-max subtraction
cancels in the normalization; the [-30,30] clip and 1e5/1e-10 guards are
provably inactive here), and the partial output projection over its 384
head-dims. Host sums the two head-group partials per batch and adds b_out.

Token permutation: queries with (t % 64) % 3 == 0 are zeroed by the
reference's load mask, making their attention output mean(v) per head.
Tokens are permuted live-first so the 672 live queries are contiguous.
A pinned zero q-column at index 672 yields exp=1 everywhere, so its
accT row carries [sum(v) | 1024] = the masked-query output.

Pipeline: the PV matmuls run transposed (stationary = exp-tile slice,
moving = v), producing accT[queries, dims] so softmax normalization is a
per-partition-scalar DVE op; normalized head pairs are transposed back
on the PE with an identity matmul. Each head-phase interleaves
scores(h, kt) / pv(h-1, kt) / filler work (v projection, q/k projection
passes, pair transposes) kt-by-kt so PE and ACT run concurrently.
PSUM start=True wipes a whole bank, so every bank hosts exactly one
start=True writer.
"""
import numpy as np
import ml_dtypes

import concourse.bass as bass
import concourse.mybir as mybir
import concourse.tile as tile
from concourse import bacc
from concourse.bass import ts
from concourse.bass_utils import run_bass_kernel_spmd

B, T, E = 4, 1024, 768
H, D = 12, 64
HG = 6                  # heads per group
GD = HG * D             # 384 head-dims per group
BLOCK_M = 64
LIVE = 672              # tokens with (t % BLOCK_M) % 3 != 0
MASK = T - LIVE         # 352
NQ = LIVE + 1           # live queries + pinned zero column (masked-mean)
SCALE = 1.0 / 8.0       # 1/sqrt(64)
QT_N = 6                # query chunks: 5 x 128 + 1 x 33

BF16 = mybir.dt.bfloat16
F32 = mybir.dt.float32

_perm = None
_nc = None


def _perm_live_first():
    t = np.arange(T)
    m = (t % BLOCK_M) % 3 == 0
    return np.concatenate([t[~m], t[m]])


def _qt_slices():
    out = []
    for qt in range(QT_N):
        q0 = qt * 128
        q1 = min(q0 + 128, NQ)
        out.append((q0, q1))
    return out


def _build_bass(debug=False):
    nc = bacc.Bacc()
    # w layout: [128 part, 9 chunks, 6 ek, 128]; chunks 0-2 = q, 3-5 = k,
    # 6-8 = v; per (partition, chunk) the 6*128 elements are contiguous so
    # chunked DMAs run at full descriptor size.
    xT_d = nc.dram_tensor("xT", [E, T], BF16, kind="ExternalInput")
    w_d = nc.dram_tensor("w", [128, 9, 6, 128], BF16, kind="ExternalInput")
    woT_d = nc.dram_tensor("woT", [GD, E], BF16, kind="ExternalInput")
    idn_d = nc.dram_tensor("idn", [128, 128], BF16, kind="ExternalInput")
    out_d = nc.dram_tensor("out", [T, E], BF16, kind="ExternalOutput")

    qts = _qt_slices()

    with tile.TileContext(nc) as tc:
        with tc.tile_pool(name="singles", bufs=1) as singles:
            xT_sb = singles.tile([128, 6, T], BF16)
            w_sb = singles.tile([128, 9, 6, 128], BF16)
            woT_sb = singles.tile([128, 3, E], BF16)
            idn_sb = singles.tile([128, 128], BF16)
            qT_sb = singles.tile([128, 3, 676], BF16)  # col 672 pinned 0
            kT_sb = singles.tile([128, 3, T], BF16)
            v_sb = singles.tile([128, 8, HG * (D + 1)], BF16)  # per-head v|1
            attnT_sb = singles.tile([128, 3, T], BF16)
            ones_sb = singles.tile([128, MASK], BF16)

            # --- input DMAs in dependency order (device executes in order)
            nc.sync.dma_start(out=w_sb[:, 0, :, :], in_=w_d[:, 0, :, :])  # q0
            nc.sync.dma_start(out=w_sb[:, 3, :, :], in_=w_d[:, 3, :, :])  # k0
            for ek in range(6):
                nc.sync.dma_start(out=xT_sb[:, ek, :],
                                  in_=xT_d[128 * ek:128 * (ek + 1), :])
            nc.sync.dma_start(out=w_sb[:, 6:9, :, :], in_=w_d[:, 6:9, :, :])
            nc.sync.dma_start(out=w_sb[:, 1, :, :], in_=w_d[:, 1, :, :])  # q1
            nc.sync.dma_start(out=w_sb[:, 4, :, :], in_=w_d[:, 4, :, :])  # k1
            nc.sync.dma_start(out=w_sb[:, 2, :, :], in_=w_d[:, 2, :, :])  # q2
            nc.sync.dma_start(out=w_sb[:, 5, :, :], in_=w_d[:, 5, :, :])  # k2
            nc.sync.dma_start(
                out=woT_sb, in_=woT_d[:, :].rearrange("(c p) t -> p c t", p=128))
            nc.sync.dma_start(out=idn_sb, in_=idn_d[:, :])

            nc.vector.memset(ones_sb, 1.0)
            v_ones = v_sb[:, :, :].rearrange(
                "p a (h e) -> p a h e", e=D + 1)[:, :, :, D:D + 1]
            nc.vector.memset(v_ones, 1.0)
            nc.vector.memset(qT_sb[:, :, 672:676], 0.0)

            acc_tiles = {}
            an_tiles = {}
            pps = {}
            pools = {}

            def proj_pass(mt, half, on_act=False):
                """One 1-bank projection pass: q (mt 0-2) or k (mt 3-5),
                half 0 = cols 0:512, half 1 = cols 512:end."""
                isq = mt < 3
                ncols = LIVE if isq else T
                s0, s1 = (0, 512) if half == 0 else (512, ncols)
                ps = pools["pj"].tile([128, 512], F32, tag="pj", name="pjps")
                for ek in range(6):
                    nc.tensor.matmul(ps[:, 0:s1 - s0],
                                     w_sb[:, mt, ek, :],
                                     xT_sb[:, ek, s0:s1],
                                     start=(ek == 0), stop=(ek == 5))
                dst = (qT_sb[:, mt, s0:s1] if isq
                       else kT_sb[:, mt - 3, s0:s1])
                if on_act:
                    nc.scalar.copy(dst, ps[:, 0:s1 - s0])
                else:
                    nc.vector.tensor_copy(dst, ps[:, 0:s1 - s0])

            def scores_exp(h, kt):
                c, po = h // 2, 64 * (h % 2)
                kh = kT_sb[po:po + 64, c, ts(kt, 128)]
                qh = qT_sb[po:po + 64, c, :]
                sT = pools["sT"].tile([128, T], F32, tag="sT", name="sT")
                nc.tensor.matmul(sT[:, 0:512], kh, qh[:, 0:512],
                                 start=True, stop=True)
                nc.tensor.matmul(sT[:, 512:NQ], kh, qh[:, 512:NQ],
                                 start=True, stop=True)
                pp = pools["pp"].tile([128, NQ], BF16, tag="pp", name="pp")
                nc.scalar.activation(pp[:, 0:NQ], sT[:, 0:NQ],
                                     mybir.ActivationFunctionType.Exp,
                                     scale=SCALE)
                pps[(h, kt)] = pp

            def v_proj(kt):
                vp = pools["v"].tile([128, GD], F32, tag="vps", name="vps")
                for ek in range(6):
                    nc.tensor.matmul(vp,
                                     xT_sb[:, ek, ts(kt, 128)],
                                     w_sb[:, 6:9, ek, :],
                                     start=(ek == 0), stop=(ek == 5))
                dst = v_sb[:, kt, :].rearrange(
                    "p (h e) -> p h e", e=D + 1)[:, :, 0:D]
                nc.vector.tensor_copy(
                    dst, vp[:, :].rearrange("p (h d) -> p h d", d=D))

            def open_head(h):
                acc_tiles[h] = pools["acc"].tile([128, QT_N * 65], F32,
                                                 tag="acc", name="acc")

            def open_pair(c):
                for qt in range(QT_N):
                    an_tiles[(c, qt)] = pools["an"].tile(
                        [128, 128], BF16, tag="an", name="an")

            def pv_t(h, kt):
                # start=True wipes the whole PSUM bank for partitions
                # [0, roundup(M, 64)), so only the first matmul of the bank
                # carries it; it zeroes all six qt regions at once.
                pp = pps.pop((h, kt))
                acc = acc_tiles[h]
                vh = v_sb[:, kt, h * (D + 1):(h + 1) * (D + 1)]
                for qt, (q0, q1) in enumerate(qts):
                    nc.tensor.matmul(acc[0:q1 - q0, qt * 65:qt * 65 + 65],
                                     pp[:, q0:q1], vh,
                                     start=(kt == 0 and qt == 0),
                                     stop=(kt == 7),
                                     skip_group_check=True)

            def norm(h):
                c, po = h // 2, 64 * (h % 2)
                acc = acc_tiles[h]
                rd = pools["rd"].tile([128, QT_N], F32, tag="rd", name="rd")
                dcols = acc[:, :].rearrange(
                    "p (q e) -> p q e", e=65)[:, :, 64:65]
                nc.vector.reciprocal(rd, dcols)
                for qt, (q0, q1) in enumerate(qts):
                    an = an_tiles[(c, qt)]
                    nc.vector.tensor_scalar_mul(
                        an[0:q1 - q0, po:po + 64],
                        acc[0:q1 - q0, qt * 65:qt * 65 + 64],
                        rd[0:q1 - q0, qt:qt + 1])

            def pair_qt_finish(c, qt):
                """Transpose one normalized [128q, 128d] pair tile back to
                [dims, tokens] and copy into attnT."""
                q0, q1 = qts[qt]
                an = an_tiles.pop((c, qt))
                tp = pools["tp"].tile([128, 128], BF16, tag="tp", name="tp")
                nc.tensor.transpose(tp, an, idn_sb)
                nc.vector.tensor_copy(attnT_sb[:, c, q0:q1], tp[:, 0:q1 - q0])

            def pair_fill(c):
                mv = pools["rd"].tile([128, 1], F32, tag="mv", name="mv")
                nc.vector.tensor_copy(mv, attnT_sb[:, c, 672:673])
                nc.vector.tensor_scalar_mul(attnT_sb[:, c, LIVE:T],
                                            ones_sb, mv)

            def out_proj(tt, eng):
                ps = pools["o"].tile([128, E], F32, tag="ops", name="ops")
                for s0, s1 in ((0, 512), (512, E)):
                    for c3 in range(3):
                        nc.tensor.matmul(ps[:, s0:s1],
                                         attnT_sb[:, c3, ts(tt, 128)],
                                         woT_sb[:, c3, s0:s1],
                                         start=(c3 == 0), stop=(c3 == 2))
                ob = pools["ob"].tile([128, E], BF16, tag="ob", name="ob")
                if eng == 0:
                    nc.scalar.copy(ob, ps)
                else:
                    nc.vector.tensor_copy(ob, ps)
                dma_eng = (nc.sync, nc.scalar, nc.gpsimd)[tt % 3]
                dma_eng.dma_start(out=out_d[ts(tt, 128), :], in_=ob)

            def phase(h, fillers, post=()):
                """8 slots: scores(h, kt) + pv(h-1, kt) + one filler each."""
                fill = list(fillers)
                for kt in range(8):
                    scores_exp(h, kt)
                    if h > 0:
                        pv_t(h - 1, kt)
                    if kt < len(fill):
                        fill[kt]()
                for f in fill[8:]:
                    f()
                for f in post:
                    f()

            with tc.tile_pool(name="sT_ps", bufs=2, space="PSUM") as sT_pool, \
                 tc.tile_pool(name="pj_ps", bufs=1, space="PSUM") as pj_pool, \
                 tc.tile_pool(name="acc_ps", bufs=2, space="PSUM") as acc_pool, \
                 tc.tile_pool(name="pp", bufs=16) as pp_pool, \
                 tc.tile_pool(name="an", bufs=12) as an_pool, \
                 tc.tile_pool(name="rd", bufs=4) as rd_pool:
                pools.update(sT=sT_pool, pj=pj_pool, acc=acc_pool,
                             pp=pp_pool, an=an_pool, rd=rd_pool)

                # pipeline head: project q0, k0 (copies on ACT: it is idle)
                for half in (0, 1):
                    proj_pass(0, half, on_act=True)
                for half in (0, 1):
                    proj_pass(3, half, on_act=True)

                with tc.tile_pool(name="v_ps", bufs=1, space="PSUM") as v_pool:
                    pools["v"] = v_pool
                    open_pair(0)
                    phase(0, [lambda kt=kt: v_proj(kt) for kt in range(4)]
                             + [lambda: proj_pass(1, 0),
                                lambda: proj_pass(1, 1)])
                    open_head(0)
                    phase(1, [lambda kt=kt: v_proj(kt) for kt in range(4, 8)]
                             + [lambda: proj_pass(4, 0),
                                lambda: proj_pass(4, 1)],
                          post=[lambda: norm(0)])

                with tc.tile_pool(name="tp_ps", bufs=1, space="PSUM") as tp_pool:
                    pools["tp"] = tp_pool
                    open_pair(1)
                    open_head(1)
                    phase(2, [lambda: proj_pass(2, 0),
                              lambda: proj_pass(2, 1)],
                          post=[lambda: norm(1)])
                    open_head(2)
                    phase(3, [lambda: proj_pass(5, 0),
                              lambda: proj_pass(5, 1)]
                             + [lambda qt=qt: pair_qt_finish(0, qt)
                                for qt in range(QT_N)],
                          post=[lambda: norm(2), lambda: pair_fill(0)])
                    open_pair(2)
                    open_head(3)
                    phase(4, [], post=[lambda: norm(3)])
                    open_head(4)
                    phase(5, [lambda qt=qt: pair_qt_finish(1, qt)
                              for qt in range(QT_N)],
                          post=[lambda: norm(4), lambda: pair_fill(1)])

                    # drain: pv(5) + per-qt norm/transpose pipelined
                    open_head(5)
                    for kt in range(8):
                        pv_t(5, kt)
                    if debug:
                        dacc = nc.dram_tensor("dbg_acc5", [128, QT_N * 65],
                                              F32, kind="ExternalOutput")
                        dacc_sb = singles.tile([128, QT_N * 65], F32,
                                               name="dacc_sb")
                        nc.vector.tensor_copy(dacc_sb, acc_tiles[5])
                        nc.sync.dma_start(out=dacc[:, :], in_=dacc_sb)
                    norm(5)
                    for qt in range(QT_N):
                        pair_qt_finish(2, qt)
                    pair_fill(2)

            if debug:
                for nm, t, sh in (("dbg_q", qT_sb, [128, 3, 676]),
                                  ("dbg_k", kT_sb, [128, 3, T]),
                                  ("dbg_v", v_sb, [128, 8, HG * (D + 1)]),
                                  ("dbg_a", attnT_sb, [128, 3, T])):
                    dd = nc.dram_tensor(nm, sh, BF16, kind="ExternalOutput")
                    nc.sync.dma_start(out=dd[:, :, :], in_=t[:, :, :])

            # ---- output projection (partial over this group's 384 dims)
            with tc.tile_pool(name="o_ps", bufs=3, space="PSUM") as o_pool, \
                 tc.tile_pool(name="ob", bufs=4) as ob_pool:
                pools.update(o=o_pool, ob=ob_pool)
                for tt in range(8):
                    out_proj(tt, tt % 2)

    nc.finalize()
    return nc


def _get_bass():
    global _nc
    if _nc is None:
        _nc = _build_bass()
    return _nc


def kernel(x, idx, struct_embed, w_qkv, w_out, b_out):
    global _perm
    if _perm is None:
        _perm = _perm_live_first()
    perm = _perm

    x = np.asarray(x, dtype=np.float32)
    idx = np.asarray(idx)
    struct_embed = np.asarray(struct_embed, dtype=np.float32)
    w_qkv = np.asarray(w_qkv, dtype=np.float32)
    w_out = np.asarray(w_out, dtype=np.float32)
    b_out = np.asarray(b_out, dtype=np.float32)

    sid = ((idx == 1) * 1 + (idx == 2) * 2 + (idx == 3) * 3)  # [B,T]
    xs = x + struct_embed[sid]                                # fold on host

    bf = ml_dtypes.bfloat16
    idn = np.eye(128, dtype=bf)
    in_maps = []
    for core in range(8):
        b, g = core // 2, core % 2
        wg = np.concatenate([w_qkv[g * GD:(g + 1) * GD],
                             w_qkv[E + g * GD:E + (g + 1) * GD],
                             w_qkv[2 * E + g * GD:2 * E + (g + 1) * GD]],
                            axis=0)                           # [3GD, E] q|k|v
        wgT = np.ascontiguousarray(wg.T)                      # [E, 3GD]
        wpack = wgT.reshape(6, 128, 9, 128).transpose(1, 2, 0, 3)
        in_maps.append({
            "xT": np.ascontiguousarray(xs[b].T[:, perm]).astype(bf),
            "w": np.ascontiguousarray(wpack).astype(bf),
            "woT": np.ascontiguousarray(
                w_out[:, g * GD:(g + 1) * GD].T).astype(bf),
            "idn": idn,
        })

    res = run_bass_kernel_spmd(_get_bass(), in_maps, core_ids=list(range(8)))

    inv = np.empty(T, dtype=np.int64)
    inv[perm] = np.arange(T)
    out = np.empty((B, T, E), dtype=np.float32)
    for b in range(B):
        acc = (res.results[2 * b]["out"].astype(np.float32)
               + res.results[2 * b + 1]["out"].astype(np.float32))
        out[b] = acc[inv] + b_out[None, :]
    return out


# revision 16
# speedup vs baseline: 1.6178x; 1.0064x over previous
"""Trainium2 Bass kernel for nn_MultiHeadAttention_8074538516581.

Sharding: 8 cores = batch(4) x head-group(2 groups of 6 heads).
Each core computes, for its (b, g): qkv projection for its 6 heads
(struct-embed folded into x on the host), per-head attention with the
reference's exact semantics (q/k rounded to bf16; the row-max subtraction
cancels in the normalization; the [-30,30] clip and 1e5/1e-10 guards are
provably inactive here), and the partial output projection over its 384
head-dims. Host sums the two head-group partials per batch and adds b_out.

Token permutation: queries with (t % 64) % 3 == 0 are zeroed by the
reference's load mask, making their attention output mean(v) per head.
Tokens are permuted live-first so the 672 live queries are contiguous.
A pinned zero q-column at index 672 yields exp=1 everywhere, so its
accT row carries [sum(v) | 1024] = the masked-query output.

Pipeline: the PV matmuls run transposed (stationary = exp-tile slice,
moving = v), producing accT[queries, dims] so softmax normalization is a
per-partition-scalar DVE op; normalized head pairs are transposed back
on the PE with an identity matmul. Each head-phase interleaves
scores(h, kt) / pv(h-1, kt) / filler work (v projection, q/k projection
passes, pair transposes) kt-by-kt so PE and ACT run concurrently.
PSUM start=True wipes a whole bank, so every bank hosts exactly one
start=True writer.
"""
import numpy as np
import ml_dtypes

import concourse.bass as bass
import concourse.mybir as mybir
import concourse.tile as tile
from concourse import bacc
from concourse.bass import ts
from concourse.bass_utils import run_bass_kernel_spmd

B, T, E = 4, 1024, 768
H, D = 12, 64
HG = 6                  # heads per group
GD = HG * D             # 384 head-dims per group
BLOCK_M = 64
LIVE = 672              # tokens with (t % BLOCK_M) % 3 != 0
MASK = T - LIVE         # 352
NQ = LIVE + 1           # live queries + pinned zero column (masked-mean)
SCALE = 1.0 / 8.0       # 1/sqrt(64)
QT_N = 6                # query chunks: 5 x 128 + 1 x 33

BF16 = mybir.dt.bfloat16
F32 = mybir.dt.float32

_perm = None
_nc = None


def _perm_live_first():
    t = np.arange(T)
    m = (t % BLOCK_M) % 3 == 0
    return np.concatenate([t[~m], t[m]])


def _qt_slices():
    out = []
    for qt in range(QT_N):
        q0 = qt * 128
        q1 = min(q0 + 128, NQ)
        out.append((q0, q1))
    return out


def _build_bass(debug=False):
    nc = bacc.Bacc()
    # w layout: [128 part, 9 chunks, 6 ek, 128]; chunks 0-2 = q, 3-5 = k,
    # 6-8 = v; per (partition, chunk) the 6*128 elements are contiguous so
    # chunked DMAs run at full descriptor size.
    xT_d = nc.dram_tensor("xT", [E, T], BF16, kind="ExternalInput")
    w_d = nc.dram_tensor("w", [128, 9, 6, 128], BF16, kind="ExternalInput")
    woT_d = nc.dram_tensor("woT", [GD, E], BF16, kind="ExternalInput")
    idn_d = nc.dram_tensor("idn", [128, 128], BF16, kind="ExternalInput")
    out_d = nc.dram_tensor("out", [T, E], BF16, kind="ExternalOutput")

    qts = _qt_slices()

    with tile.TileContext(nc) as tc:
        with tc.tile_pool(name="singles", bufs=1) as singles:
            xT_sb = singles.tile([128, 6, T], BF16)
            w_sb = singles.tile([128, 9, 6, 128], BF16)
            woT_sb = singles.tile([128, 3, E], BF16)
            idn_sb = singles.tile([128, 128], BF16)
            qT_sb = singles.tile([128, 3, 676], BF16)  # col 672 pinned 0
            kT_sb = singles.tile([128, 3, T], BF16)
            v_sb = singles.tile([128, 8, HG * (D + 1)], BF16)  # per-head v|1
            attnT_sb = singles.tile([128, 3, T], BF16)
            ones_sb = singles.tile([128, MASK], BF16)

            # --- input DMAs in dependency order (device executes in order)
            nc.sync.dma_start(out=w_sb[:, 0, :, :], in_=w_d[:, 0, :, :])  # q0
            nc.sync.dma_start(out=w_sb[:, 3, :, :], in_=w_d[:, 3, :, :])  # k0
            # live-token columns first: q projection and early scores only
            # need tokens 0:672, so compute starts before masked cols land
            for ek in range(6):
                nc.sync.dma_start(out=xT_sb[:, ek, 0:LIVE],
                                  in_=xT_d[128 * ek:128 * (ek + 1), 0:LIVE])
            for ek in range(6):
                nc.sync.dma_start(out=xT_sb[:, ek, LIVE:T],
                                  in_=xT_d[128 * ek:128 * (ek + 1), LIVE:T])
            nc.sync.dma_start(out=w_sb[:, 6:9, :, :], in_=w_d[:, 6:9, :, :])
            nc.sync.dma_start(out=w_sb[:, 1, :, :], in_=w_d[:, 1, :, :])  # q1
            nc.sync.dma_start(out=w_sb[:, 4, :, :], in_=w_d[:, 4, :, :])  # k1
            nc.sync.dma_start(out=w_sb[:, 2, :, :], in_=w_d[:, 2, :, :])  # q2
            nc.sync.dma_start(out=w_sb[:, 5, :, :], in_=w_d[:, 5, :, :])  # k2
            nc.sync.dma_start(
                out=woT_sb, in_=woT_d[:, :].rearrange("(c p) t -> p c t", p=128))
            nc.sync.dma_start(out=idn_sb, in_=idn_d[:, :])

            nc.vector.memset(ones_sb, 1.0)
            v_ones = v_sb[:, :, :].rearrange(
                "p a (h e) -> p a h e", e=D + 1)[:, :, :, D:D + 1]
            nc.vector.memset(v_ones, 1.0)
            nc.vector.memset(qT_sb[:, :, 672:676], 0.0)
            # preload the Exp table during the DMA window so the first real
            # exp doesn't pay the ~1.3us LoadActFuncSet
            warm = singles.tile([1, 1], F32, name="warm")
            nc.scalar.activation(warm, ones_sb[0:1, 0:1],
                                 mybir.ActivationFunctionType.Exp)

            acc_tiles = {}
            an_tiles = {}
            pps = {}
            pools = {}

            def proj_pass(mt, half, on_act=False):
                """One 1-bank projection pass: q (mt 0-2) or k (mt 3-5),
                half 0 = cols 0:512, half 1 = cols 512:end."""
                isq = mt < 3
                ncols = LIVE if isq else T
                s0, s1 = (0, 512) if half == 0 else (512, ncols)
                ps = pools["pj"].tile([128, 512], F32, tag="pj", name="pjps")
                for ek in range(6):
                    nc.tensor.matmul(ps[:, 0:s1 - s0],
                                     w_sb[:, mt, ek, :],
                                     xT_sb[:, ek, s0:s1],
                                     start=(ek == 0), stop=(ek == 5))
                dst = (qT_sb[:, mt, s0:s1] if isq
                       else kT_sb[:, mt - 3, s0:s1])
                if on_act:
                    nc.scalar.copy(dst, ps[:, 0:s1 - s0])
                else:
                    nc.vector.tensor_copy(dst, ps[:, 0:s1 - s0])

            def scores_exp(h, kt):
                c, po = h // 2, 64 * (h % 2)
                kh = kT_sb[po:po + 64, c, ts(kt, 128)]
                qh = qT_sb[po:po + 64, c, :]
                sT = pools["sT"].tile([128, T], F32, tag="sT", name="sT")
                nc.tensor.matmul(sT[:, 0:512], kh, qh[:, 0:512],
                                 start=True, stop=True)
                nc.tensor.matmul(sT[:, 512:NQ], kh, qh[:, 512:NQ],
                                 start=True, stop=True)
                pp = pools["pp"].tile([128, NQ], BF16, tag="pp", name="pp")
                nc.scalar.activation(pp[:, 0:NQ], sT[:, 0:NQ],
                                     mybir.ActivationFunctionType.Exp,
                                     scale=SCALE)
                pps[(h, kt)] = pp

            def v_proj(kt):
                vp = pools["v"].tile([128, GD], F32, tag="vps", name="vps")
                for ek in range(6):
                    nc.tensor.matmul(vp,
                                     xT_sb[:, ek, ts(kt, 128)],
                                     w_sb[:, 6:9, ek, :],
                                     start=(ek == 0), stop=(ek == 5))
                dst = v_sb[:, kt, :].rearrange(
                    "p (h e) -> p h e", e=D + 1)[:, :, 0:D]
                nc.vector.tensor_copy(
                    dst, vp[:, :].rearrange("p (h d) -> p h d", d=D))

            def open_head(h):
                acc_tiles[h] = pools["acc"].tile([128, QT_N * 65], F32,
                                                 tag="acc", name="acc")

            def open_pair(c):
                for qt in range(QT_N):
                    an_tiles[(c, qt)] = pools["an"].tile(
                        [128, 128], BF16, tag="an", name="an")

            def pv_t(h, kt):
                # start=True wipes the whole PSUM bank for partitions
                # [0, roundup(M, 64)), so only the first matmul of the bank
                # carries it; it zeroes all six qt regions at once.
                pp = pps.pop((h, kt))
                acc = acc_tiles[h]
                vh = v_sb[:, kt, h * (D + 1):(h + 1) * (D + 1)]
                for qt, (q0, q1) in enumerate(qts):
                    nc.tensor.matmul(acc[0:q1 - q0, qt * 65:qt * 65 + 65],
                                     pp[:, q0:q1], vh,
                                     start=(kt == 0 and qt == 0),
                                     stop=(kt == 7),
                                     skip_group_check=True)

            def norm(h):
                c, po = h // 2, 64 * (h % 2)
                acc = acc_tiles[h]
                rd = pools["rd"].tile([128, QT_N], F32, tag="rd", name="rd")
                dcols = acc[:, :].rearrange(
                    "p (q e) -> p q e", e=65)[:, :, 64:65]
                nc.vector.reciprocal(rd, dcols)
                for qt, (q0, q1) in enumerate(qts):
                    an = an_tiles[(c, qt)]
                    nc.vector.tensor_scalar_mul(
                        an[0:q1 - q0, po:po + 64],
                        acc[0:q1 - q0, qt * 65:qt * 65 + 64],
                        rd[0:q1 - q0, qt:qt + 1])

            def pair_qt_finish(c, qt):
                """Transpose one normalized [128q, 128d] pair tile back to
                [dims, tokens] and copy into attnT."""
                q0, q1 = qts[qt]
                an = an_tiles.pop((c, qt))
                tp = pools["tp"].tile([128, 128], BF16, tag="tp", name="tp")
                nc.tensor.transpose(tp, an, idn_sb)
                nc.vector.tensor_copy(attnT_sb[:, c, q0:q1], tp[:, 0:q1 - q0])

            def pair_fill(c):
                mv = pools["rd"].tile([128, 1], F32, tag="mv", name="mv")
                nc.vector.tensor_copy(mv, attnT_sb[:, c, 672:673])
                nc.vector.tensor_scalar_mul(attnT_sb[:, c, LIVE:T],
                                            ones_sb, mv)

            def out_proj(tt, eng):
                ps = pools["o"].tile([128, E], F32, tag="ops", name="ops")
                for s0, s1 in ((0, 512), (512, E)):
                    for c3 in range(3):
                        nc.tensor.matmul(ps[:, s0:s1],
                                         attnT_sb[:, c3, ts(tt, 128)],
                                         woT_sb[:, c3, s0:s1],
                                         start=(c3 == 0), stop=(c3 == 2))
                ob = pools["ob"].tile([128, E], BF16, tag="ob", name="ob")
                if eng == 0:
                    nc.scalar.copy(ob, ps)
                else:
                    nc.vector.tensor_copy(ob, ps)
                dma_eng = (nc.sync, nc.scalar, nc.gpsimd)[tt % 3]
                dma_eng.dma_start(out=out_d[ts(tt, 128), :], in_=ob)

            def phase(h, fillers, post=()):
                """8 slots: scores(h, kt) + pv(h-1, kt) + one filler each."""
                fill = list(fillers)
                for kt in range(8):
                    scores_exp(h, kt)
                    if h > 0:
                        pv_t(h - 1, kt)
                    if kt < len(fill):
                        fill[kt]()
                for f in fill[8:]:
                    f()
                for f in post:
                    f()

            with tc.tile_pool(name="acc_ps", bufs=2, space="PSUM") as acc_pool, \
                 tc.tile_pool(name="pp", bufs=16) as pp_pool, \
                 tc.tile_pool(name="an", bufs=12) as an_pool, \
                 tc.tile_pool(name="rd", bufs=4) as rd_pool:
                pools.update(acc=acc_pool, pp=pp_pool, an=an_pool, rd=rd_pool)

                with tc.tile_pool(name="sT_ps", bufs=2, space="PSUM") as sT_pool, \
                     tc.tile_pool(name="pj_ps", bufs=1, space="PSUM") as pj_pool:
                    pools.update(sT=sT_pool, pj=pj_pool)

                    # pipeline head: project q0, k0 (copies on ACT: idle)
                    for half in (0, 1):
                        proj_pass(0, half, on_act=True)
                    for half in (0, 1):
                        proj_pass(3, half, on_act=True)

                    with tc.tile_pool(name="v_ps", bufs=1,
                                      space="PSUM") as v_pool:
                        pools["v"] = v_pool
                        open_pair(0)
                        phase(0, [lambda kt=kt: v_proj(kt) for kt in range(4)]
                                 + [lambda: proj_pass(1, 0),
                                    lambda: proj_pass(1, 1)])
                        open_head(0)
                        phase(1, [lambda kt=kt: v_proj(kt)
                                  for kt in range(4, 8)]
                                 + [lambda: proj_pass(4, 0),
                                    lambda: proj_pass(4, 1)],
                              post=[lambda: norm(0)])

                    with tc.tile_pool(name="tp_ps", bufs=1,
                                      space="PSUM") as tp_pool:
                        pools["tp"] = tp_pool
                        open_pair(1)
                        open_head(1)
                        phase(2, [lambda: proj_pass(2, 0),
                                  lambda: proj_pass(2, 1)],
                              post=[lambda: norm(1)])
                        open_head(2)
                        phase(3, [lambda: proj_pass(5, 0),
                                  lambda: proj_pass(5, 1)]
                                 + [lambda qt=qt: pair_qt_finish(0, qt)
                                    for qt in range(QT_N)],
                              post=[lambda: norm(2), lambda: pair_fill(0)])
                        open_pair(2)
                        open_head(3)
                        phase(4, [], post=[lambda: norm(3)])
                        open_head(4)
                        phase(5, [lambda qt=qt: pair_qt_finish(1, qt)
                                  for qt in range(QT_N)],
                              post=[lambda: norm(4), lambda: pair_fill(1)])
                        open_head(5)

                # sT/pj freed; drain + output projection share the banks
                with tc.tile_pool(name="o_ps", bufs=2, space="PSUM") as o_pool, \
                     tc.tile_pool(name="ob", bufs=4) as ob_pool:
                    pools.update(o=o_pool, ob=ob_pool, tp=o_pool)
                    for kt in range(8):
                        pv_t(5, kt)
                    if debug:
                        dacc = nc.dram_tensor("dbg_acc5", [128, QT_N * 65],
                                              F32, kind="ExternalOutput")
                        dacc_sb = singles.tile([128, QT_N * 65], F32,
                                               name="dacc_sb")
                        nc.vector.tensor_copy(dacc_sb, acc_tiles[5])
                        nc.sync.dma_start(out=dacc[:, :], in_=dacc_sb)
                    # last head's norm / transpose / out-proj interleaved:
                    # out-proj tile qt only needs attnT chunk-2 cols < 128qt+128
                    c, po = 2, 64
                    acc = acc_tiles[5]
                    rd = rd_pool.tile([128, QT_N], F32, tag="rd", name="rd")
                    nc.vector.reciprocal(rd, acc[:, :].rearrange(
                        "p (q e) -> p q e", e=65)[:, :, 64:65])
                    for qt, (q0, q1) in enumerate(qts):
                        an = an_tiles[(c, qt)]
                        nc.vector.tensor_scalar_mul(
                            an[0:q1 - q0, po:po + 64],
                            acc[0:q1 - q0, qt * 65:qt * 65 + 64],
                            rd[0:q1 - q0, qt:qt + 1])
                        pair_qt_finish(c, qt)
                        if qt < 5:
                            out_proj(qt, qt % 2)
                    pair_fill(2)
                    for tt in (5, 6, 7):
                        out_proj(tt, tt % 2)

            if debug:
                for nm, t, sh in (("dbg_q", qT_sb, [128, 3, 676]),
                                  ("dbg_k", kT_sb, [128, 3, T]),
                                  ("dbg_v", v_sb, [128, 8, HG * (D + 1)]),
                                  ("dbg_a", attnT_sb, [128, 3, T])):
                    dd = nc.dram_tensor(nm, sh, BF16, kind="ExternalOutput")
                    nc.sync.dma_start(out=dd[:, :, :], in_=t[:, :, :])

    nc.finalize()
    return nc


def _get_bass():
    global _nc
    if _nc is None:
        _nc = _build_bass()
    return _nc


def kernel(x, idx, struct_embed, w_qkv, w_out, b_out):
    global _perm
    if _perm is None:
        _perm = _perm_live_first()
    perm = _perm

    x = np.asarray(x, dtype=np.float32)
    idx = np.asarray(idx)
    struct_embed = np.asarray(struct_embed, dtype=np.float32)
    w_qkv = np.asarray(w_qkv, dtype=np.float32)
    w_out = np.asarray(w_out, dtype=np.float32)
    b_out = np.asarray(b_out, dtype=np.float32)

    sid = ((idx == 1) * 1 + (idx == 2) * 2 + (idx == 3) * 3)  # [B,T]
    xs = x + struct_embed[sid]                                # fold on host

    bf = ml_dtypes.bfloat16
    idn = np.eye(128, dtype=bf)
    in_maps = []
    for core in range(8):
        b, g = core // 2, core % 2
        wg = np.concatenate([w_qkv[g * GD:(g + 1) * GD],
                             w_qkv[E + g * GD:E + (g + 1) * GD],
                             w_qkv[2 * E + g * GD:2 * E + (g + 1) * GD]],
                            axis=0)                           # [3GD, E] q|k|v
        wgT = np.ascontiguousarray(wg.T)                      # [E, 3GD]
        wpack = wgT.reshape(6, 128, 9, 128).transpose(1, 2, 0, 3)
        in_maps.append({
            "xT": np.ascontiguousarray(xs[b].T[:, perm]).astype(bf),
            "w": np.ascontiguousarray(wpack).astype(bf),
            "woT": np.ascontiguousarray(
                w_out[:, g * GD:(g + 1) * GD].T).astype(bf),
            "idn": idn,
        })

    res = run_bass_kernel_spmd(_get_bass(), in_maps, core_ids=list(range(8)))

    inv = np.empty(T, dtype=np.int64)
    inv[perm] = np.arange(T)
    out = np.empty((B, T, E), dtype=np.float32)
    for b in range(B):
        acc = (res.results[2 * b]["out"].astype(np.float32)
               + res.results[2 * b + 1]["out"].astype(np.float32))
        out[b] = acc[inv] + b_out[None, :]
    return out
